# revision 13
# baseline (speedup 1.0000x reference)
"""Locally-connected 2D layer on 8 Trainium2 NeuronCores.

Problem: x[128,3,64,64] f32, per-position weights W[60,60,32,75], bias b[60,60,32]
  out[b,o,y,x] = sum_k patches[b,y,x,k] * W[y,x,o,k] + b[y,x,o],  k=(c,dy,dx)

Strategy (spatial sharding over output rows, 8 rows/core, memory-regime):
  - Groups of 4 consecutive x positions share one full-array matmul: the
    contraction is the UNION of the 4 patch windows, planes (c, dy, dx') with
    dx' in 0..7 -> 5*3*8 = 120 partitions (padded to 128).  The stationary
    [128, 128] holds all 4 positions' weights ((j,o) columns, structural
    zeros where dx'-j is outside 0..4), so each [128, 128] matmul output is
    fully useful: out[(j,o), b] for 4 x-positions at once.  15 matmuls per
    row, 120 per core, N=128 streaming.  Bias is added on the host.
  - dy uses a mod-5 ring of row-slots (24 planes each) with the per-row dy
    rotation folded into the host-side W slab (np.roll).  The ring is kept in
    TWO column-generations (even rows read gen A, odd rows gen B): after row
    k finishes, two whole slots of gen k%2 are refreshed (SBUF->SBUF from the
    staged future rows) for row k+2, so refills have a full row of slack and
    matmuls never wait on them.
  - Queue split matches measured rates (gpsimd SWDGE ~300GB/s, both HWDGE
    queues combined ~120GB/s): gpsimd carries the initial ring fill, the W
    slab and most stores; sync/scalar carry the small future-row slabs, the
    ring refreshes and the remaining stores.
  - Output is bf16 (host upcasts).  PSUM evacuation alternates vector/scalar.
  - 20 full-size dummy matmuls on a scratch tile run during the initial fill
    so the PE HAM clock-gate is warm when real matmuls start.
"""

import numpy as np

B, C, H, WIDTH = 128, 3, 64, 64
KH = KW = 5
RY = RX = 60
O = 32
NCORES = 8
RPC = 8             # output rows computed per core (8*8=64, last 4 dropped)
INR = RPC + KH - 1  # 12 input rows per core
PADH = NCORES * RPC + KH - 1  # 68
NG = 15             # groups of 4 x-positions per row
NPL = C * 8         # 24 planes per ring slot (c, dx' in 0..7)
KP = KH * NPL       # 120 contraction partitions (before padding)
FU = NG * B         # 1920 free elems per plane (g, b)
CHUNKS = ((0, 4), (4, 4), (8, 4), (12, 3))  # (first group, n groups) per PSUM chunk
NWARM = 20

_cache = {}


def _build():
    import concourse.bass as bass
    import concourse.bacc as bacc
    import concourse.tile as tile
    import concourse.mybir as mybir

    f32 = mybir.dt.float32
    din = mybir.dt.bfloat16
    nc = bacc.Bacc("TRN2", target_bir_lowering=False, debug=False,
                   num_devices=NCORES)
    ui_d = nc.dram_tensor("ui", [KP, FU], din, kind="ExternalInput")
    ufa_d = nc.dram_tensor("ufa", [KP, FU], din, kind="ExternalInput")
    ufb_d = nc.dram_tensor("ufb", [2 * NPL, FU], din, kind="ExternalInput")
    w_d = nc.dram_tensor("w", [128, RPC * NG * B], din, kind="ExternalInput")
    oc_d = nc.dram_tensor("oc", [RPC, 4, O, NG, B], din, kind="ExternalOutput")

    with tile.TileContext(nc) as tc:
        with (
            tc.tile_pool(name="const", bufs=1) as cpool,
            tc.tile_pool(name="os", bufs=8) as opool,
            tc.tile_pool(name="ps", bufs=4, space=bass.MemorySpace.PSUM) as ppool,
            tc.tile_pool(name="pw", bufs=1, space=bass.MemorySpace.PSUM) as wpool,
        ):
            xp = cpool.tile([128, 2 * FU], din)       # ring, 2 generations
            ufa = cpool.tile([KP, FU], din)           # future rows 5-9
            ufb = cpool.tile([2 * NPL, FU], din)      # future rows 10-11
            ws = cpool.tile([128, RPC * NG * B], din)
            dm = cpool.tile([128, 640], din)          # warmup operand

            # pad partitions 120-127 with finite values (weights there are 0)
            nc.vector.memset(xp[96:128, :], 1.0)

            nc.gpsimd.dma_start(xp[0:KP, 0:FU], ui_d[:])      # gen A rows 0-4
            nc.gpsimd.dma_start(ws[:, 0:FU], w_d[:, 0:FU])    # W row 0 first
            for p in range(4):  # W rows (1,2) (3,4) (5,6) (7) on SWDGE
                c0 = (2 * p + 1) * FU
                c1 = min((2 * p + 3) * FU, RPC * FU)
                nc.gpsimd.dma_start(ws[:, c0:c1], w_d[:, c0:c1])
            nc.scalar.dma_start(ufa[:], ufa_d[:])
            nc.sync.dma_start(ufb[:], ufb_d[:])

            # gen B init: rows 1-4 live in the same slots as gen A; slot 0
            # takes row 5 from the future slab
            nc.sync.dma_start(xp[NPL:KP, FU:2 * FU], xp[NPL:KP, 0:FU])
            nc.sync.dma_start(xp[0:NPL, FU:2 * FU], ufa[0:NPL, :])

            # PE warmup: keep the array genuinely busy (full K/M) during the
            # fill so HAM un-throttles the PE clock before the first real
            # matmul; HAM ignores low-activity (tiny-K) matmuls
            nc.vector.memset(dm[:], 1.0)
            pw = wpool.tile([128, 512], f32)
            for _ in range(NWARM):
                nc.tensor.matmul(pw[:, :], dm[:, 0:128], dm[:, 128:640])

            for k in range(RPC):
                gofs = (k % 2) * FU
                ot = opool.tile([128, FU], din)
                oc_k = oc_d[k].rearrange("j o g b -> (j o) (g b)")
                for ci, (g0, gn) in enumerate(CHUNKS):
                    pt = ppool.tile([128, 4 * B], f32)
                    for gg in range(gn):
                        g = g0 + gg
                        nc.tensor.matmul(
                            pt[:, gg * B:(gg + 1) * B],
                            ws[:, (k * NG + g) * B:(k * NG + g + 1) * B],
                            xp[:, gofs + g * B:gofs + (g + 1) * B],
                        )
                    if ci % 2 == 0:
                        nc.vector.tensor_copy(
                            ot[:, g0 * B:(g0 + gn) * B], pt[:, :gn * B])
                    else:
                        nc.scalar.copy(
                            ot[:, g0 * B:(g0 + gn) * B], pt[:, :gn * B])
                # ring refresh for row k+2 (same generation): slots k%5 and
                # (k+1)%5 take input rows k+5 and k+6
                if k < RPC - 2:
                    for s, r in ((k % KH, k + KH), ((k + 1) % KH, k + KH + 1)):
                        src = (ufa[(r - KH) * NPL:(r - KH + 1) * NPL, :]
                               if r < 2 * KH else
                               ufb[(r - 2 * KH) * NPL:(r - 2 * KH + 1) * NPL, :])
                        nc.sync.dma_start(
                            xp[s * NPL:(s + 1) * NPL, gofs:gofs + FU], src)
                if k == RPC - 1:  # split the last store to shorten the tail
                    nc.gpsimd.dma_start(oc_k[:, 0:8 * B], ot[:, 0:8 * B])
                    nc.gpsimd.dma_start(oc_k[:, 8 * B:FU], ot[:, 8 * B:FU])
                else:
                    st_eng = nc.gpsimd if k < 4 else (nc.sync, nc.scalar,
                                                      nc.sync)[k - 4]
                    st_eng.dma_start(oc_k[:], ot[:])

    nc.compile()
    return nc


def _get_nc():
    if "nc" not in _cache:
        _cache["nc"] = _build()
    return _cache["nc"]


def _prep_inputs(x, W, b):
    import ml_dtypes
    bf = ml_dtypes.bfloat16
    x = np.asarray(x, np.float32)
    W = np.asarray(W, np.float32)
    xh = np.zeros((PADH, C, WIDTH, B), np.float32)
    xh[:H] = x.transpose(2, 1, 3, 0)  # [row, c, w, batch]
    # union planes: U[row, (c,dx'), (g,b)] = xh[row, c, 4g+dx', b], dx' in 0..7
    U = np.zeros((PADH, C, 8, NG, B), np.float32)
    for dxp in range(8):
        U[:, :, dxp] = xh[:, :, dxp::4][:, :, :NG]
    U = U.reshape(PADH, NPL, FU).astype(bf)

    W5 = W.reshape(RY, RX, O, C, KH, KW)
    in_maps = []
    for i in range(NCORES):
        nk = min(RPC, RY - RPC * i)
        W5c = np.zeros((RPC, NG, 4, O, C, KH, KW), np.float32)
        W5c[:nk] = W5[RPC * i:RPC * i + nk].reshape(nk, NG, 4, O, C, KH, KW)
        A = W5c.transpose(5, 4, 6, 0, 1, 2, 3)  # [dy, c, dx, k, g, j, o]
        D = np.zeros((KH, C, 8, RPC, NG, 4, O), np.float32)
        for j in range(4):
            D[:, :, j:j + KW, :, :, j, :] = A[:, :, :, :, :, j, :]
        S = np.empty_like(D)  # slot rm holds dy=(rm-k)%5 -> roll dy by k
        for k in range(RPC):
            S[:, :, :, k] = np.roll(D[:, :, :, k], k, axis=0)
        wslab = np.zeros((128, RPC * NG * 4 * O), np.float32)
        wslab[:KP] = S.reshape(KP, -1)

        Uc = U[RPC * i:RPC * i + INR]  # [12, 24, FU]
        in_maps.append({
            "ui": np.ascontiguousarray(Uc[:KH].reshape(KP, FU)),
            "ufa": np.ascontiguousarray(Uc[KH:2 * KH].reshape(KP, FU)),
            "ufb": np.ascontiguousarray(Uc[2 * KH:].reshape(2 * NPL, FU)),
            "w": wslab.astype(bf),
        })
    return in_maps


def kernel(x, W, b):
    from concourse.bass_utils import run_bass_kernel_spmd

    nc = _get_nc()
    in_maps = _prep_inputs(x, W, b)
    br = run_bass_kernel_spmd(nc, in_maps, list(range(NCORES)),
                              **_cache.get("run_kwargs", {}))
    _cache["last_run"] = br
    oc = np.stack([np.asarray(br.results[i]["oc"]) for i in range(NCORES)])
    oc = oc.reshape(NCORES * RPC, 4, O, NG, B).astype(np.float32)
    out = oc.transpose(4, 2, 0, 3, 1).reshape(B, O, NCORES * RPC, RX)
    out = out[:, :, :RY, :] + np.asarray(b, np.float32).transpose(2, 0, 1)[None]
    return np.ascontiguousarray(out)


# revision 18
# speedup vs baseline: 1.0880x; 1.0880x over previous
"""Locally-connected 2D layer on 8 Trainium2 NeuronCores.

Problem: x[128,3,64,64] f32, per-position weights W[60,60,32,75], bias b[60,60,32]
  out[b,o,y,x] = sum_k patches[b,y,x,k] * W[y,x,o,k] + b[y,x,o],  k=(c,dy,dx)

Strategy (spatial sharding over output rows, 8 rows/core, memory-regime):
  - Groups of 4 consecutive x positions share one full-array matmul: the
    contraction is the UNION of the 4 patch windows, planes (c, dy, dx') with
    dx' in 0..7 -> 5*3*8 = 120 partitions (padded to 128).  The stationary
    [128, 128] holds all 4 positions' weights ((j,o) columns, structural
    zeros where dx'-j is outside 0..4), so each [128, 128] matmul output is
    fully useful: out[(j,o), b] for 4 x-positions at once.  15 matmuls per
    row, 120 per core, N=128 streaming.  Bias is added on the host.
  - dy uses a mod-5 ring of row-slots (24 planes each) with the per-row dy
    rotation folded into the host-side W slab (np.roll).  The ring is kept in
    TWO column-generations (even rows read gen A, odd rows gen B): after row
    k finishes, two whole slots of gen k%2 are refreshed (SBUF->SBUF from the
    staged future rows) for row k+2, so refills have a full row of slack and
    matmuls never wait on them.
  - Queue split matches measured rates (gpsimd SWDGE ~300GB/s, both HWDGE
    queues combined ~120GB/s): gpsimd carries the initial ring fill, the W
    slab and most stores; sync/scalar carry the small future-row slabs, the
    ring refreshes and the remaining stores.
  - Output is bf16 (host upcasts).  PSUM evacuation alternates vector/scalar.
  - 20 full-size dummy matmuls on a scratch tile run during the initial fill
    so the PE HAM clock-gate is warm when real matmuls start.
"""

import numpy as np

B, C, H, WIDTH = 128, 3, 64, 64
KH = KW = 5
RY = RX = 60
O = 32
NCORES = 8
RPC = 8             # output rows computed per core (8*8=64, last 4 dropped)
INR = RPC + KH - 1  # 12 input rows per core
PADH = NCORES * RPC + KH - 1  # 68
NG = 15             # groups of 4 x-positions per row
NPL = C * 8         # 24 planes per ring slot (c, dx' in 0..7)
KP = KH * NPL       # 120 contraction partitions (before padding)
FU = NG * B         # 1920 free elems per plane (g, b)
CHUNKS = ((0, 4), (4, 4), (8, 4), (12, 3))  # (first group, n groups) per PSUM chunk
NWARM = 20

_cache = {}


def _build():
    import concourse.bass as bass
    import concourse.bacc as bacc
    import concourse.tile as tile
    import concourse.mybir as mybir

    f32 = mybir.dt.float32
    din = mybir.dt.bfloat16
    nc = bacc.Bacc("TRN2", target_bir_lowering=False, debug=False,
                   num_devices=NCORES)
    ui_d = nc.dram_tensor("ui", [KP, FU], din, kind="ExternalInput")
    uib_d = nc.dram_tensor("uib", [KP, FU], din, kind="ExternalInput")
    ufa_d = nc.dram_tensor("ufa", [KP, FU], din, kind="ExternalInput")
    ufb_d = nc.dram_tensor("ufb", [2 * NPL, FU], din, kind="ExternalInput")
    w_d = nc.dram_tensor("w", [128, RPC * NG * B], din, kind="ExternalInput")
    oc_d = nc.dram_tensor("oc", [RPC, 4, O, NG, B], din, kind="ExternalOutput")

    with tile.TileContext(nc) as tc:
        with (
            tc.tile_pool(name="const", bufs=1) as cpool,
            tc.tile_pool(name="os", bufs=8) as opool,
            tc.tile_pool(name="ps", bufs=4, space=bass.MemorySpace.PSUM) as ppool,
            tc.tile_pool(name="pw", bufs=1, space=bass.MemorySpace.PSUM) as wpool,
        ):
            xp = cpool.tile([KP, 2 * FU], din)        # ring, 2 generations
            ufa = cpool.tile([KP, FU], din)           # future rows 5-9
            ufb = cpool.tile([2 * NPL, FU], din)      # future rows 10-11
            ws = cpool.tile([128, RPC * NG * B], din)
            dm = cpool.tile([128, 640], din)          # warmup operand

            nc.gpsimd.dma_start(xp[:, 0:FU], ui_d[:])         # gen A rows 0-4
            nc.gpsimd.dma_start(ws[:, 0:FU], w_d[:, 0:FU])    # W row 0 first
            for p in range(4):  # W rows (1,2) (3,4) (5,6) (7) on SWDGE
                c0 = (2 * p + 1) * FU
                c1 = min((2 * p + 3) * FU, RPC * FU)
                nc.gpsimd.dma_start(ws[:, c0:c1], w_d[:, c0:c1])
            nc.scalar.dma_start(xp[:, FU:2 * FU], uib_d[:])   # gen B rows 1-5
            nc.scalar.dma_start(ufa[:], ufa_d[:])
            nc.sync.dma_start(ufb[:], ufb_d[:])

            # PE warmup: keep the array genuinely busy (full K/M) during the
            # fill so HAM un-throttles the PE clock before the first real
            # matmul; HAM ignores low-activity (tiny-K) matmuls
            nc.vector.memset(dm[:], 1.0)
            pw = wpool.tile([128, 512], f32)
            for _ in range(NWARM):
                nc.tensor.matmul(pw[:, :], dm[:, 0:128], dm[:, 128:640])

            for k in range(RPC):
                gofs = (k % 2) * FU
                ot = opool.tile([128, FU], din)
                oc_k = oc_d[k].rearrange("j o g b -> (j o) (g b)")
                for ci, (g0, gn) in enumerate(CHUNKS):
                    pt = ppool.tile([128, 4 * B], f32)
                    for gg in range(gn):
                        g = g0 + gg
                        nc.tensor.matmul(
                            pt[:, gg * B:(gg + 1) * B],
                            ws[0:KP, (k * NG + g) * B:(k * NG + g + 1) * B],
                            xp[:, gofs + g * B:gofs + (g + 1) * B],
                        )
                    if ci % 2 == 0:
                        nc.vector.tensor_copy(
                            ot[:, g0 * B:(g0 + gn) * B], pt[:, :gn * B])
                    else:
                        nc.scalar.copy(
                            ot[:, g0 * B:(g0 + gn) * B], pt[:, :gn * B])
                # ring refresh for row k+2 (same generation): slots k%5 and
                # (k+1)%5 take input rows k+5 and k+6
                if k < RPC - 2:
                    for ri, (s, r) in enumerate(
                            ((k % KH, k + KH), ((k + 1) % KH, k + KH + 1))):
                        src = (ufa[(r - KH) * NPL:(r - KH + 1) * NPL, :]
                               if r < 2 * KH else
                               ufb[(r - 2 * KH) * NPL:(r - 2 * KH + 1) * NPL, :])
                        (nc.sync if ri == 0 else nc.scalar).dma_start(
                            xp[s * NPL:(s + 1) * NPL, gofs:gofs + FU], src)
                if k == RPC - 1:  # split the last store to shorten the tail
                    nc.gpsimd.dma_start(oc_k[:, 0:8 * B], ot[:, 0:8 * B])
                    nc.gpsimd.dma_start(oc_k[:, 8 * B:FU], ot[:, 8 * B:FU])
                else:
                    st_eng = nc.gpsimd if k < 4 else (nc.sync, nc.scalar,
                                                      nc.sync)[k - 4]
                    st_eng.dma_start(oc_k[:], ot[:])

    nc.compile()
    return nc


def _get_nc():
    if "nc" not in _cache:
        _cache["nc"] = _build()
    return _cache["nc"]


def _prep_inputs(x, W, b):
    import ml_dtypes
    bf = ml_dtypes.bfloat16
    x = np.asarray(x, np.float32)
    W = np.asarray(W, np.float32)
    xh = np.zeros((PADH, C, WIDTH, B), np.float32)
    xh[:H] = x.transpose(2, 1, 3, 0)  # [row, c, w, batch]
    # union planes: U[row, (c,dx'), (g,b)] = xh[row, c, 4g+dx', b], dx' in 0..7
    U = np.zeros((PADH, C, 8, NG, B), np.float32)
    for dxp in range(8):
        U[:, :, dxp] = xh[:, :, dxp::4][:, :, :NG]
    U = U.reshape(PADH, NPL, FU).astype(bf)

    W5 = W.reshape(RY, RX, O, C, KH, KW)
    in_maps = []
    for i in range(NCORES):
        nk = min(RPC, RY - RPC * i)
        W5c = np.zeros((RPC, NG, 4, O, C, KH, KW), np.float32)
        W5c[:nk] = W5[RPC * i:RPC * i + nk].reshape(nk, NG, 4, O, C, KH, KW)
        A = W5c.transpose(5, 4, 6, 0, 1, 2, 3)  # [dy, c, dx, k, g, j, o]
        D = np.zeros((KH, C, 8, RPC, NG, 4, O), np.float32)
        for j in range(4):
            D[:, :, j:j + KW, :, :, j, :] = A[:, :, :, :, :, j, :]
        S = np.empty_like(D)  # slot rm holds dy=(rm-k)%5 -> roll dy by k
        for k in range(RPC):
            S[:, :, :, k] = np.roll(D[:, :, :, k], k, axis=0)
        wslab = np.zeros((128, RPC * NG * 4 * O), np.float32)
        wslab[:KP] = S.reshape(KP, -1)

        Uc = U[RPC * i:RPC * i + INR]  # [12, 24, FU]
        uib = Uc[[5, 1, 2, 3, 4]]  # gen B init: slot s holds row s (row 5 at 0)
        in_maps.append({
            "ui": np.ascontiguousarray(Uc[:KH].reshape(KP, FU)),
            "uib": np.ascontiguousarray(uib.reshape(KP, FU)),
            "ufa": np.ascontiguousarray(Uc[KH:2 * KH].reshape(KP, FU)),
            "ufb": np.ascontiguousarray(Uc[2 * KH:].reshape(2 * NPL, FU)),
            "w": wslab.astype(bf),
        })
    return in_maps


def kernel(x, W, b):
    from concourse.bass_utils import run_bass_kernel_spmd

    nc = _get_nc()
    in_maps = _prep_inputs(x, W, b)
    br = run_bass_kernel_spmd(nc, in_maps, list(range(NCORES)),
                              **_cache.get("run_kwargs", {}))
    _cache["last_run"] = br
    oc = np.stack([np.asarray(br.results[i]["oc"]) for i in range(NCORES)])
    oc = oc.reshape(NCORES * RPC, 4, O, NG, B).astype(np.float32)
    out = oc.transpose(4, 2, 0, 3, 1).reshape(B, O, NCORES * RPC, RX)
    out = out[:, :, :RY, :] + np.asarray(b, np.float32).transpose(2, 0, 1)[None]
    return np.ascontiguousarray(out)


# revision 20
# speedup vs baseline: 1.1044x; 1.0151x over previous
"""Locally-connected 2D layer on 8 Trainium2 NeuronCores.

Problem: x[128,3,64,64] f32, per-position weights W[60,60,32,75], bias b[60,60,32]
  out[b,o,y,x] = sum_k patches[b,y,x,k] * W[y,x,o,k] + b[y,x,o],  k=(c,dy,dx)

Strategy (spatial sharding over output rows, 8 rows/core, memory-regime):
  - Groups of 4 consecutive x positions share one full-array matmul: the
    contraction is the UNION of the 4 patch windows, planes (c, dy, dx') with
    dx' in 0..7 -> 5*3*8 = 120 partitions (padded to 128).  The stationary
    [128, 128] holds all 4 positions' weights ((j,o) columns, structural
    zeros where dx'-j is outside 0..4), so each [128, 128] matmul output is
    fully useful: out[(j,o), b] for 4 x-positions at once.  15 matmuls per
    row, 120 per core, N=128 streaming.  Bias is added on the host.
  - dy uses a mod-5 ring of row-slots (24 planes each) with the per-row dy
    rotation folded into the host-side W slab (np.roll).  The ring is kept in
    TWO column-generations (even rows read gen A, odd rows gen B): after row
    k finishes, two whole slots of gen k%2 are refreshed (SBUF->SBUF from the
    staged future rows) for row k+2, so refills have a full row of slack and
    matmuls never wait on them.
  - Queue split matches measured rates (gpsimd SWDGE ~300GB/s, both HWDGE
    queues combined ~120GB/s): gpsimd carries the initial ring fill, the W
    slab and most stores; sync/scalar carry the small future-row slabs, the
    ring refreshes and the remaining stores.
  - Output is bf16 (host upcasts).  PSUM evacuation alternates vector/scalar.
  - 20 full-size dummy matmuls on a scratch tile run during the initial fill
    so the PE HAM clock-gate is warm when real matmuls start.
"""

import numpy as np

B, C, H, WIDTH = 128, 3, 64, 64
KH = KW = 5
RY = RX = 60
O = 32
NCORES = 8
RPC = 8             # output rows computed per core (8*8=64, last 4 dropped)
INR = RPC + KH - 1  # 12 input rows per core
PADH = NCORES * RPC + KH - 1  # 68
NG = 15             # groups of 4 x-positions per row
NPL = C * 8         # 24 planes per ring slot (c, dx' in 0..7)
KP = KH * NPL       # 120 contraction partitions (before padding)
FU = NG * B         # 1920 free elems per plane (g, b)
CHUNKS = ((0, 4), (4, 4), (8, 4), (12, 3))  # (first group, n groups) per PSUM chunk
NWARM = 20

_cache = {}


def _build():
    import concourse.bass as bass
    import concourse.bacc as bacc
    import concourse.tile as tile
    import concourse.mybir as mybir

    f32 = mybir.dt.float32
    din = mybir.dt.bfloat16
    nc = bacc.Bacc("TRN2", target_bir_lowering=False, debug=False,
                   num_devices=NCORES)
    ui_d = nc.dram_tensor("ui", [KP, FU], din, kind="ExternalInput")
    uib_d = nc.dram_tensor("uib", [KP, FU], din, kind="ExternalInput")
    ufa_d = nc.dram_tensor("ufa", [KP, FU], din, kind="ExternalInput")
    ufb_d = nc.dram_tensor("ufb", [2 * NPL, FU], din, kind="ExternalInput")
    w_d = nc.dram_tensor("w", [128, RPC * NG * B], din, kind="ExternalInput")
    oc_d = nc.dram_tensor("oc", [RPC, 4, O, NG, B], din, kind="ExternalOutput")

    with tile.TileContext(nc) as tc:
        with (
            tc.tile_pool(name="const", bufs=1) as cpool,
            tc.tile_pool(name="os", bufs=8) as opool,
            tc.tile_pool(name="ps", bufs=4, space=bass.MemorySpace.PSUM) as ppool,
            tc.tile_pool(name="pw", bufs=1, space=bass.MemorySpace.PSUM) as wpool,
        ):
            xp = cpool.tile([KP, 2 * FU], din)        # ring, 2 generations
            ufa = cpool.tile([KP, FU], din)           # future rows 5-9
            ufb = cpool.tile([2 * NPL, FU], din)      # future rows 10-11
            ws = cpool.tile([128, RPC * NG * B], din)
            dm = cpool.tile([128, 640], din)          # warmup operand

            nc.gpsimd.dma_start(xp[:, 0:FU], ui_d[:])         # gen A rows 0-4
            nc.gpsimd.dma_start(ws[:, 0:FU], w_d[:, 0:FU])    # W row 0 first
            for p in range(4):  # W rows (1,2) (3,4) (5,6) (7) on SWDGE
                c0 = (2 * p + 1) * FU
                c1 = min((2 * p + 3) * FU, RPC * FU)
                nc.gpsimd.dma_start(ws[:, c0:c1], w_d[:, c0:c1])
            nc.scalar.dma_start(xp[:, FU:2 * FU], uib_d[:])   # gen B rows 1-5
            nc.scalar.dma_start(ufa[:], ufa_d[:])
            nc.sync.dma_start(ufb[:], ufb_d[:])

            # PE warmup: keep the array genuinely busy (full K/M) during the
            # fill so HAM un-throttles the PE clock before the first real
            # matmul; HAM ignores low-activity (tiny-K) matmuls
            nc.vector.memset(dm[:], 1.0)
            pw = wpool.tile([128, 512], f32)
            for _ in range(NWARM):
                nc.tensor.matmul(pw[:, :], dm[:, 0:128], dm[:, 128:640])

            for k in range(RPC):
                gofs = (k % 2) * FU
                ot = opool.tile([128, FU], din)
                oc_k = oc_d[k].rearrange("j o g b -> (j o) (g b)")
                for ci, (g0, gn) in enumerate(CHUNKS):
                    pt = ppool.tile([128, 4 * B], f32)
                    for gg in range(gn):
                        g = g0 + gg
                        nc.tensor.matmul(
                            pt[:, gg * B:(gg + 1) * B],
                            ws[0:KP, (k * NG + g) * B:(k * NG + g + 1) * B],
                            xp[:, gofs + g * B:gofs + (g + 1) * B],
                        )
                    if ci % 2 == 0:
                        nc.vector.tensor_copy(
                            ot[:, g0 * B:(g0 + gn) * B], pt[:, :gn * B])
                    else:
                        nc.scalar.copy(
                            ot[:, g0 * B:(g0 + gn) * B], pt[:, :gn * B])
                # ring refresh for row k+2 (same generation): slots k%5 and
                # (k+1)%5 take input rows k+5 and k+6
                if k < RPC - 2:
                    for ri, (s, r) in enumerate(
                            ((k % KH, k + KH), ((k + 1) % KH, k + KH + 1))):
                        src = (ufa[(r - KH) * NPL:(r - KH + 1) * NPL, :]
                               if r < 2 * KH else
                               ufb[(r - 2 * KH) * NPL:(r - 2 * KH + 1) * NPL, :])
                        nc.sync.dma_start(
                            xp[s * NPL:(s + 1) * NPL, gofs:gofs + FU], src)
                if k == RPC - 1:  # split the last store to shorten the tail
                    nc.gpsimd.dma_start(oc_k[:, 0:8 * B], ot[:, 0:8 * B])
                    nc.gpsimd.dma_start(oc_k[:, 8 * B:FU], ot[:, 8 * B:FU])
                else:
                    # sync is reserved for ring refreshes (they are latency
                    # critical and must not queue behind 492KB stores)
                    st_eng = nc.gpsimd if k < 4 else nc.scalar
                    st_eng.dma_start(oc_k[:], ot[:])

    nc.compile()
    return nc


def _get_nc():
    if "nc" not in _cache:
        _cache["nc"] = _build()
    return _cache["nc"]


def _prep_inputs(x, W, b):
    import ml_dtypes
    bf = ml_dtypes.bfloat16
    x = np.asarray(x, np.float32)
    W = np.asarray(W, np.float32)
    xh = np.zeros((PADH, C, WIDTH, B), np.float32)
    xh[:H] = x.transpose(2, 1, 3, 0)  # [row, c, w, batch]
    # union planes: U[row, (c,dx'), (g,b)] = xh[row, c, 4g+dx', b], dx' in 0..7
    U = np.zeros((PADH, C, 8, NG, B), np.float32)
    for dxp in range(8):
        U[:, :, dxp] = xh[:, :, dxp::4][:, :, :NG]
    U = U.reshape(PADH, NPL, FU).astype(bf)

    W5 = W.reshape(RY, RX, O, C, KH, KW)
    in_maps = []
    for i in range(NCORES):
        nk = min(RPC, RY - RPC * i)
        W5c = np.zeros((RPC, NG, 4, O, C, KH, KW), np.float32)
        W5c[:nk] = W5[RPC * i:RPC * i + nk].reshape(nk, NG, 4, O, C, KH, KW)
        A = W5c.transpose(5, 4, 6, 0, 1, 2, 3)  # [dy, c, dx, k, g, j, o]
        D = np.zeros((KH, C, 8, RPC, NG, 4, O), np.float32)
        for j in range(4):
            D[:, :, j:j + KW, :, :, j, :] = A[:, :, :, :, :, j, :]
        S = np.empty_like(D)  # slot rm holds dy=(rm-k)%5 -> roll dy by k
        for k in range(RPC):
            S[:, :, :, k] = np.roll(D[:, :, :, k], k, axis=0)
        wslab = np.zeros((128, RPC * NG * 4 * O), np.float32)
        wslab[:KP] = S.reshape(KP, -1)

        Uc = U[RPC * i:RPC * i + INR]  # [12, 24, FU]
        uib = Uc[[5, 1, 2, 3, 4]]  # gen B init: slot s holds row s (row 5 at 0)
        in_maps.append({
            "ui": np.ascontiguousarray(Uc[:KH].reshape(KP, FU)),
            "uib": np.ascontiguousarray(uib.reshape(KP, FU)),
            "ufa": np.ascontiguousarray(Uc[KH:2 * KH].reshape(KP, FU)),
            "ufb": np.ascontiguousarray(Uc[2 * KH:].reshape(2 * NPL, FU)),
            "w": wslab.astype(bf),
        })
    return in_maps


def kernel(x, W, b):
    from concourse.bass_utils import run_bass_kernel_spmd

    nc = _get_nc()
    in_maps = _prep_inputs(x, W, b)
    br = run_bass_kernel_spmd(nc, in_maps, list(range(NCORES)),
                              **_cache.get("run_kwargs", {}))
    _cache["last_run"] = br
    oc = np.stack([np.asarray(br.results[i]["oc"]) for i in range(NCORES)])
    oc = oc.reshape(NCORES * RPC, 4, O, NG, B).astype(np.float32)
    out = oc.transpose(4, 2, 0, 3, 1).reshape(B, O, NCORES * RPC, RX)
    out = out[:, :, :RY, :] + np.asarray(b, np.float32).transpose(2, 0, 1)[None]
    return np.ascontiguousarray(out)


# revision 22
# speedup vs baseline: 1.1205x; 1.0146x over previous
"""Locally-connected 2D layer on 8 Trainium2 NeuronCores.

Problem: x[128,3,64,64] f32, per-position weights W[60,60,32,75], bias b[60,60,32]
  out[b,o,y,x] = sum_k patches[b,y,x,k] * W[y,x,o,k] + b[y,x,o],  k=(c,dy,dx)

Strategy (spatial sharding over output rows, 8 rows/core, memory-regime):
  - Groups of 4 consecutive x positions share one full-array matmul: the
    contraction is the UNION of the 4 patch windows, planes (c, dy, dx') with
    dx' in 0..7 -> 5*3*8 = 120 partitions (padded to 128).  The stationary
    [128, 128] holds all 4 positions' weights ((j,o) columns, structural
    zeros where dx'-j is outside 0..4), so each [128, 128] matmul output is
    fully useful: out[(j,o), b] for 4 x-positions at once.  15 matmuls per
    row, 120 per core, N=128 streaming.  Bias is added on the host.
  - dy uses a mod-5 ring of row-slots (24 planes each) with the per-row dy
    rotation folded into the host-side W slab (np.roll).  The ring is kept in
    TWO column-generations (even rows read gen A, odd rows gen B): after row
    k finishes, two whole slots of gen k%2 are refreshed (SBUF->SBUF from the
    staged future rows) for row k+2, so refills have a full row of slack and
    matmuls never wait on them.
  - Queue split matches measured rates (gpsimd SWDGE ~300GB/s, both HWDGE
    queues combined ~120GB/s): gpsimd carries the initial ring fill, the W
    slab and most stores; sync/scalar carry the small future-row slabs, the
    ring refreshes and the remaining stores.
  - Output is bf16 (host upcasts).  PSUM evacuation alternates vector/scalar.
  - 20 full-size dummy matmuls on a scratch tile run during the initial fill
    so the PE HAM clock-gate is warm when real matmuls start.
"""

import numpy as np

B, C, H, WIDTH = 128, 3, 64, 64
KH = KW = 5
RY = RX = 60
O = 32
NCORES = 8
RPC = 8             # output rows computed per core (8*8=64, last 4 dropped)
INR = RPC + KH - 1  # 12 input rows per core
PADH = NCORES * RPC + KH - 1  # 68
NG = 15             # groups of 4 x-positions per row
NPL = C * 8         # 24 planes per ring slot (c, dx' in 0..7)
KP = KH * NPL       # 120 contraction partitions (before padding)
FU = NG * B         # 1920 free elems per plane (g, b)
CHUNKS = ((0, 4), (4, 4), (8, 4), (12, 3))  # (first group, n groups) per PSUM chunk
NWARM = 20

_cache = {}


def _build():
    import concourse.bass as bass
    import concourse.bacc as bacc
    import concourse.tile as tile
    import concourse.mybir as mybir

    f32 = mybir.dt.float32
    din = mybir.dt.bfloat16
    nc = bacc.Bacc("TRN2", target_bir_lowering=False, debug=False,
                   num_devices=NCORES)
    ui_d = nc.dram_tensor("ui", [KP, FU], din, kind="ExternalInput")
    uib_d = nc.dram_tensor("uib", [KP, FU], din, kind="ExternalInput")
    ufa_d = nc.dram_tensor("ufa", [KP, FU], din, kind="ExternalInput")
    ufb_d = nc.dram_tensor("ufb", [2 * NPL, FU], din, kind="ExternalInput")
    w_d = nc.dram_tensor("w", [128, RPC * NG * B], din, kind="ExternalInput")
    oc_d = nc.dram_tensor("oc", [RPC, 4, O, NG, B], din, kind="ExternalOutput")

    with tile.TileContext(nc) as tc:
        with (
            tc.tile_pool(name="const", bufs=1) as cpool,
            tc.tile_pool(name="os", bufs=8) as opool,
            tc.tile_pool(name="ps", bufs=4, space=bass.MemorySpace.PSUM) as ppool,
            tc.tile_pool(name="pw", bufs=1, space=bass.MemorySpace.PSUM) as wpool,
        ):
            xp = cpool.tile([KP, 2 * FU], din)        # ring, 2 generations
            ufa = cpool.tile([KP, FU], din)           # future rows 5-9
            ufb = cpool.tile([2 * NPL, FU], din)      # future rows 10-11
            ws = cpool.tile([128, RPC * NG * B], din)
            dm = cpool.tile([128, 640], din)          # warmup operand

            nc.gpsimd.dma_start(xp[:, 0:FU], ui_d[:])         # gen A rows 0-4
            nc.gpsimd.dma_start(ws[:, 0:FU], w_d[:, 0:FU])    # W row 0 first
            for p in range(4):  # W rows (1,2) (3,4) (5,6) (7) on SWDGE
                c0 = (2 * p + 1) * FU
                c1 = min((2 * p + 3) * FU, RPC * FU)
                nc.gpsimd.dma_start(ws[:, c0:c1], w_d[:, c0:c1])
            nc.scalar.dma_start(xp[:, FU:2 * FU], uib_d[:])   # gen B rows 1-5
            nc.scalar.dma_start(ufa[:], ufa_d[:])
            nc.sync.dma_start(ufb[:], ufb_d[:])

            # PE warmup: keep the array genuinely busy (full K/M) during the
            # fill so HAM un-throttles the PE clock before the first real
            # matmul; HAM ignores low-activity (tiny-K) matmuls
            nc.vector.memset(dm[:], 1.0)
            pw = wpool.tile([128, 512], f32)
            for _ in range(NWARM):
                nc.tensor.matmul(pw[:, :], dm[:, 0:128], dm[:, 128:640])

            for k in range(RPC):
                gofs = (k % 2) * FU
                ot = opool.tile([128, FU], din)
                oc_k = oc_d[k].rearrange("j o g b -> (j o) (g b)")
                for ci, (g0, gn) in enumerate(CHUNKS):
                    pt = ppool.tile([128, 4 * B], f32)
                    for gg in range(gn):
                        g = g0 + gg
                        nc.tensor.matmul(
                            pt[:, gg * B:(gg + 1) * B],
                            ws[0:KP, (k * NG + g) * B:(k * NG + g + 1) * B],
                            xp[:, gofs + g * B:gofs + (g + 1) * B],
                        )
                    if ci % 2 == 0:
                        nc.vector.tensor_copy(
                            ot[:, g0 * B:(g0 + gn) * B], pt[:, :gn * B])
                    else:
                        nc.scalar.copy(
                            ot[:, g0 * B:(g0 + gn) * B], pt[:, :gn * B])
                # ring refresh for row k+2 (same generation): slots k%5 and
                # (k+1)%5 take input rows k+5 and k+6
                if k < RPC - 2:
                    for ri, (s, r) in enumerate(
                            ((k % KH, k + KH), ((k + 1) % KH, k + KH + 1))):
                        src = (ufa[(r - KH) * NPL:(r - KH + 1) * NPL, :]
                               if r < 2 * KH else
                               ufb[(r - 2 * KH) * NPL:(r - 2 * KH + 1) * NPL, :])
                        nc.gpsimd.dma_start(
                            xp[s * NPL:(s + 1) * NPL, gofs:gofs + FU], src)
                if k == RPC - 1:  # split the last store to shorten the tail
                    nc.gpsimd.dma_start(oc_k[:, 0:8 * B], ot[:, 0:8 * B])
                    nc.gpsimd.dma_start(oc_k[:, 8 * B:FU], ot[:, 8 * B:FU])
                else:
                    st_eng = nc.gpsimd if k < 4 else (nc.scalar, nc.scalar,
                                                      nc.sync)[k - 4]
                    st_eng.dma_start(oc_k[:], ot[:])

    nc.compile()
    return nc


def _get_nc():
    if "nc" not in _cache:
        _cache["nc"] = _build()
    return _cache["nc"]


def _prep_inputs(x, W, b):
    import ml_dtypes
    bf = ml_dtypes.bfloat16
    x = np.asarray(x, np.float32)
    W = np.asarray(W, np.float32)
    xh = np.zeros((PADH, C, WIDTH, B), np.float32)
    xh[:H] = x.transpose(2, 1, 3, 0)  # [row, c, w, batch]
    # union planes: U[row, (c,dx'), (g,b)] = xh[row, c, 4g+dx', b], dx' in 0..7
    U = np.zeros((PADH, C, 8, NG, B), np.float32)
    for dxp in range(8):
        U[:, :, dxp] = xh[:, :, dxp::4][:, :, :NG]
    U = U.reshape(PADH, NPL, FU).astype(bf)

    W5 = W.reshape(RY, RX, O, C, KH, KW)
    in_maps = []
    for i in range(NCORES):
        nk = min(RPC, RY - RPC * i)
        W5c = np.zeros((RPC, NG, 4, O, C, KH, KW), np.float32)
        W5c[:nk] = W5[RPC * i:RPC * i + nk].reshape(nk, NG, 4, O, C, KH, KW)
        A = W5c.transpose(5, 4, 6, 0, 1, 2, 3)  # [dy, c, dx, k, g, j, o]
        D = np.zeros((KH, C, 8, RPC, NG, 4, O), np.float32)
        for j in range(4):
            D[:, :, j:j + KW, :, :, j, :] = A[:, :, :, :, :, j, :]
        S = np.empty_like(D)  # slot rm holds dy=(rm-k)%5 -> roll dy by k
        for k in range(RPC):
            S[:, :, :, k] = np.roll(D[:, :, :, k], k, axis=0)
        wslab = np.zeros((128, RPC * NG * 4 * O), np.float32)
        wslab[:KP] = S.reshape(KP, -1)

        Uc = U[RPC * i:RPC * i + INR]  # [12, 24, FU]
        uib = Uc[[5, 1, 2, 3, 4]]  # gen B init: slot s holds row s (row 5 at 0)
        in_maps.append({
            "ui": np.ascontiguousarray(Uc[:KH].reshape(KP, FU)),
            "uib": np.ascontiguousarray(uib.reshape(KP, FU)),
            "ufa": np.ascontiguousarray(Uc[KH:2 * KH].reshape(KP, FU)),
            "ufb": np.ascontiguousarray(Uc[2 * KH:].reshape(2 * NPL, FU)),
            "w": wslab.astype(bf),
        })
    return in_maps


def kernel(x, W, b):
    from concourse.bass_utils import run_bass_kernel_spmd

    nc = _get_nc()
    in_maps = _prep_inputs(x, W, b)
    br = run_bass_kernel_spmd(nc, in_maps, list(range(NCORES)),
                              **_cache.get("run_kwargs", {}))
    _cache["last_run"] = br
    oc = np.stack([np.asarray(br.results[i]["oc"]) for i in range(NCORES)])
    oc = oc.reshape(NCORES * RPC, 4, O, NG, B).astype(np.float32)
    out = oc.transpose(4, 2, 0, 3, 1).reshape(B, O, NCORES * RPC, RX)
    out = out[:, :, :RY, :] + np.asarray(b, np.float32).transpose(2, 0, 1)[None]
    return np.ascontiguousarray(out)
